# revision 8
# baseline (speedup 1.0000x reference)
"""Trainium2 Bass kernel for nn_DeepTransitionRNN_31928786878509.

kernel(**inputs) -> np.ndarray, matching reference.reference semantics:
a deep-transition GRU over T=512 steps, B=128 (packed-sequence masking),
D=H=256, L=4 transition layers.

Strategy: data-parallel over batch (16 rows/core on 8 cores). Each core runs
the full 512-step recurrence with the hidden state resident in SBUF in
transposed layout h^T [feat partitions x batch], weights stationary as fp16
[128,128] chunks, PSUM fp32 accumulation, sigmoid/tanh on the scalar engine.

v3 changes vs the 5.7ms baseline (which was latency-bound on the per-sub-
layer ACT/DVE tail chains, with the PE idle ~55% of each step):
  - x-projections (Wr_x/Wz_x/Wl_x/Cx/Wt) are hoisted out of the recurrence
    into per-8-step-block pre-GEMMs at moving N=128 (vs N=16 in-loop),
    interleaved into the recurrence's PE-idle gaps; their r/z/l results are
    injected into PSUM via one identity matmul per step, and Cx/Wt results
    are consumed straight from SBUF by the tail ops.
  - per-layer matmul order rr -> zz -> Tn and ACT emission order
    sigma(rr), tanh, sigma(zz) so both sigmoids hide under PE/DVE work.
  - the final blend scan is split per feature chunk so the next sub-layer's
    kc=0 matmuls overlap the second half-scan.
  - the output transpose (2 PE pairs) + DVE copy per step are replaced by a
    direct strided DMA of the hidden tile; the de-transpose happens on host.
Packed-sequence masking (out=0 for t >= lengths[b]) is applied on the host.
"""

import os
import numpy as np
from contextlib import ExitStack

import concourse.bass as bass
import concourse.bacc as bacc
import concourse.mybir as mybir
import concourse.tile as tile
from concourse.bass_utils import run_bass_kernel_spmd

f16 = mybir.dt.float16
f32 = mybir.dt.float32
AF = mybir.ActivationFunctionType
OP = mybir.AluOpType

T, B, D, H, L = 512, 128, 256, 256, 4
NCORE = 8
BS = B // NCORE
KC_D = D // 128
KC_H = H // 128
MC = H // 128
NCH = 3 * 4 * MC + 3 * KC_H * MC + 3 * L * KC_H * MC  # 84

UNROLL = 8

LAST_EXEC_NS = None  # set by kernel() when tracing is enabled


def _pack_weights(Wr, Wz, Wl, Wt, Cx, Ch, Tr, Tz, Tn):
    chunks = []

    def add(M):
        for kc in range(M.shape[0] // 128):
            for mc in range(MC):
                chunks.append(M[kc * 128:(kc + 1) * 128, mc * 128:(mc + 1) * 128])

    add(Wr); add(Wz); add(Wl); add(Cx); add(Wt); add(Ch)
    for i in range(L):
        add(Tr[i]); add(Tz[i]); add(Tn[i])
    arr = np.stack([np.asarray(c, dtype=np.float32) for c in chunks])
    arr = arr.transpose(1, 0, 2).astype(np.float16)
    ident = np.eye(128, dtype=np.float16)[:, None, :]
    arr = np.ascontiguousarray(np.concatenate([arr, ident], axis=1))
    return arr


def _pack_x_shard(x_shard):
    # [T, BS, D] -> [nblk, 128, KC_D, UNROLL*BS] fp16: per-block matmul rhs
    # with partition = feature-in-chunk and moving cols = (t, b)
    Tn_ = x_shard.shape[0]
    nblk = Tn_ // UNROLL
    y = np.asarray(x_shard, dtype=np.float16).reshape(
        nblk, UNROLL, BS, KC_D, 128)
    y = y.transpose(0, 4, 3, 1, 2)  # [nblk, 128, KC_D, UNROLL, BS]
    return np.ascontiguousarray(y.reshape(nblk, 128, KC_D, UNROLL * BS))


def _build_nc(Tsteps, unroll):
    assert Tsteps % unroll == 0
    nblk = Tsteps // unroll
    nc = bacc.Bacc(None, target_bir_lowering=False, debug=False)
    xin = nc.dram_tensor('xt', [Tsteps // unroll, 128, KC_D, unroll * BS], f16,
                         kind='ExternalInput')
    win = nc.dram_tensor('wp', [128, NCH + 1, 128], f16, kind='ExternalInput')
    oul = nc.dram_tensor('out', [Tsteps, 128, KC_H, BS], f16, kind='ExternalOutput')

    idx = {}
    pos = 0

    def reg(name, kt):
        nonlocal pos
        idx[name] = [[pos + kc * MC + mc for mc in range(MC)] for kc in range(kt)]
        pos += kt * MC

    reg('Wr', 4); reg('Wz', 4); reg('Wl', 4)
    reg('Cx', 2); reg('Wt', 2); reg('Ch', 2)
    for i in range(L):
        reg(f'Tr{i}', 2); reg(f'Tz{i}', 2); reg(f'Tn{i}', 2)
    assert pos == NCH
    ID_CHUNK = NCH

    # pre-GEMM slot order in ps_pre / xpre tiles: (r,l,z) x mc then (cx,wt) x mc
    PRE = ([('Wr', mc) for mc in range(MC)] + [('Wl', mc) for mc in range(MC)]
           + [('Wz', mc) for mc in range(MC)] + [('Cx', mc) for mc in range(MC)]
           + [('Wt', mc) for mc in range(MC)])
    NPRE = len(PRE)  # 10

    with ExitStack() as ctx:
        tc = ctx.enter_context(tile.TileContext(nc))
        wpool = ctx.enter_context(tc.tile_pool(name='w', bufs=1))
        hpool = ctx.enter_context(tc.tile_pool(name='h', bufs=1))
        spool = ctx.enter_context(tc.tile_pool(name='s', bufs=2))
        xpool = ctx.enter_context(tc.tile_pool(name='x', bufs=2))
        xppool = ctx.enter_context(tc.tile_pool(name='xp', bufs=2))
        ps_rzl_pool = ctx.enter_context(tc.tile_pool(name='ps_rzl', bufs=1, space='PSUM'))
        ps_ch_pool = ctx.enter_context(tc.tile_pool(name='ps_ch', bufs=1, space='PSUM'))
        ps_rz_pool = ctx.enter_context(tc.tile_pool(name='ps_rz', bufs=1, space='PSUM'))
        ps_n_pool = ctx.enter_context(tc.tile_pool(name='ps_n', bufs=1, space='PSUM'))
        ps_pre_pool = ctx.enter_context(tc.tile_pool(name='ps_pre', bufs=1, space='PSUM'))

        W = wpool.tile([128, NCH + 1, 128], f16)
        nc.gpsimd.dma_start(W[:], win[:])

        HTa = hpool.tile([128, KC_H, BS, 3], f16, tag='hta')
        HTb = hpool.tile([128, KC_H, BS, 3], f16, tag='htb')
        nc.gpsimd.memset(HTa[:], 0.0)
        nc.gpsimd.memset(HTb[:], 0.0)
        W4 = hpool.tile([128, KC_H, BS, 3], f32, tag='w4')
        W5 = hpool.tile([128, KC_H, BS, 3], f32, tag='w5')
        nc.gpsimd.memset(W4[:], 0.0)
        nc.gpsimd.memset(W5[:], 0.0)
        nc.gpsimd.memset(W5[:, :, :, 2], 1.0)

        def flat(ap):
            return ap.rearrange('p b j -> p (b j)')

        def mm(out_ap, name, kc, mc, rhs, start, stop):
            nc.tensor.matmul(out_ap, W[:, idx[name][kc][mc], :], rhs,
                             start=start, stop=stop)

        # ---- x pre-GEMM machinery -------------------------------------
        def emit_xblk_dma(blk):
            xb = xpool.tile([128, KC_D, unroll * BS], f16, tag='xb')
            nc.sync.dma_start(xb[:], xin[blk])
            return xb

        def pre_gemm_closures(xb, blk):
            """Closures emitting the per-block x pre-GEMM: 20 matmul pairs
            (N=128) + a PSUM->SBUF DMA. Returns (closures, xpre_tile)."""
            ps_pre = ps_pre_pool.tile([128, NPRE, unroll * BS], f32, tag='pre')
            xpre = xppool.tile([128, NPRE, unroll, BS], f16, tag='xpre',
                               name=f'xpre_{blk}')
            cls = []
            for si, (g, mc) in enumerate(PRE):
                def do(si=si, g=g, mc=mc):
                    for kc in range(KC_D):
                        mm(ps_pre[:, si], g, kc, mc, xb[:, kc],
                           kc == 0, kc == KC_D - 1)
                    nc.vector.tensor_copy(
                        xpre[:, si].rearrange('p t b -> p (t b)'),
                        ps_pre[:, si])
                cls.append(do)
            return cls, xpre

        pending = []

        def pop_pending(k=1):
            for _ in range(k):
                if pending:
                    pending.pop(0)()

        def emit_step(t, xpre, j, hcur, hother):
            # hseq[s] = tile holding h BEFORE sub-layer s (s=0 is the cell).
            hseq = [hcur if s % 2 == 0 else hother for s in range(2 + L)]

            def h_ap(tile_, kc):
                return tile_[:, kc, :, 1]

            h0 = hseq[0]
            ps_rzl = ps_rzl_pool.tile([128, 3, MC, BS], f32, tag='ps_rzl')
            ps_ch = ps_ch_pool.tile([128, MC, BS], f32, tag='ps_ch')

            # inject x-part of r/l/z preacts (one identity matmul), then
            # accumulate h-parts. r first so sigma(r) is ready early.
            nc.tensor.matmul(ps_rzl[:].rearrange('p g m b -> p (g m b)'),
                             W[:, ID_CHUNK, :], xpre[:, 0:6, j],
                             start=True, stop=False)
            for mc in range(MC):
                for kc in range(KC_H):
                    mm(ps_rzl[:, 0, mc], 'Wr', KC_D + kc, mc, h_ap(h0, kc), False,
                       mc == MC - 1 and kc == KC_H - 1)
            for mc in range(MC):
                for kc in range(KC_H):
                    mm(ps_ch[:, mc], 'Ch', kc, mc, h_ap(h0, kc),
                       mc == 0 and kc == 0, mc == MC - 1 and kc == KC_H - 1)
            for gi, g in ((1, 'Wl'), (2, 'Wz')):
                for mc in range(MC):
                    for kc in range(KC_H):
                        mm(ps_rzl[:, gi, mc], g, KC_D + kc, mc, h_ap(h0, kc), False,
                           gi == 2 and mc == MC - 1 and kc == KC_H - 1)
            pop_pending()

            s_r = spool.tile([128, MC, BS], f32, tag='s_r')
            nc.scalar.activation(s_r[:], ps_rzl[:, 0], AF.Sigmoid)
            s_l = spool.tile([128, MC, BS], f32, tag='s_l')
            nc.scalar.activation(s_l[:], ps_rzl[:, 1], AF.Sigmoid)
            u = spool.tile([128, MC, BS], f32, tag='u')
            nc.vector.tensor_tensor(u[:], s_r[:], ps_ch[:], OP.mult)
            # v = u + xCx lands in ps_ch (consumed) for the faster ACT PSUM read
            nc.vector.tensor_tensor(ps_ch[:], u[:], xpre[:, 6:8, j], OP.add)
            nnc = spool.tile([128, MC, BS], f32, tag='nnc')
            nc.scalar.activation(nnc[:], ps_ch[:], AF.Tanh)
            # 1 - sig(zpre) = sig(-zpre): cell blend h' = sig(-z)*(h - n) + n
            nc.scalar.activation(W4[:, :, :, 1], ps_rzl[:, 2], AF.Sigmoid,
                                 scale=-1.0)
            w_ = spool.tile([128, MC, BS], f32, tag='w_')
            nc.vector.tensor_tensor(w_[:], s_l[:], xpre[:, 8:10, j], OP.mult)
            nc.vector.tensor_tensor(W5[:, :, :, 1], nnc[:], w_[:], OP.add)
            nc.vector.tensor_tensor(W4[:, :, :, 0], h0[:, :, :, 1],
                                    W5[:, :, :, 1], OP.subtract)
            for c in range(KC_H):
                nc.vector.tensor_tensor_scan(
                    flat(hseq[1][:, c]), flat(W4[:, c]), flat(W5[:, c]), 1.0,
                    OP.mult, OP.add)
            pop_pending()

            for li in range(L):
                hp = hseq[1 + li]
                ps_rz = ps_rz_pool.tile([128, 2, MC, BS], f32, tag='ps_rz')
                ps_n = ps_n_pool.tile([128, MC, BS], f32, tag='ps_n')
                # rr first (its sigmoid gates the critical path), then zz,
                # then Tn; sigma(rr) hides under the zz/Tn matmuls.
                for mc in range(MC):
                    for kc in range(KC_H):
                        mm(ps_rz[:, 0, mc], f'Tr{li}', kc, mc, h_ap(hp, kc),
                           mc == 0 and kc == 0, False)
                for mc in range(MC):
                    for kc in range(KC_H):
                        mm(ps_rz[:, 1, mc], f'Tz{li}', kc, mc, h_ap(hp, kc), False,
                           mc == MC - 1 and kc == KC_H - 1)
                for mc in range(MC):
                    for kc in range(KC_H):
                        mm(ps_n[:, mc], f'Tn{li}', kc, mc, h_ap(hp, kc),
                           mc == 0 and kc == 0, mc == MC - 1 and kc == KC_H - 1)
                s_rr = spool.tile([128, MC, BS], f32, tag='s_rr')
                nc.scalar.activation(s_rr[:], ps_rz[:, 0], AF.Sigmoid)
                # m = sig(rr) * (h@Tn) lands back in ps_n for ACT PSUM read
                nc.vector.tensor_tensor(ps_n[:], s_rr[:], ps_n[:], OP.mult)
                nc.scalar.activation(W5[:, :, :, 1], ps_n[:], AF.Tanh)
                nc.scalar.activation(W4[:, :, :, 1], ps_rz[:, 1], AF.Sigmoid)
                nc.vector.tensor_tensor(W4[:, :, :, 0], hp[:, :, :, 1],
                                        W5[:, :, :, 1], OP.subtract)
                # h' = sig(zz)*(h - nn) + nn, split per chunk so the next
                # sub-layer's kc=0 matmuls overlap the second half-scan
                for c in range(KC_H):
                    nc.vector.tensor_tensor_scan(
                        flat(hseq[2 + li][:, c]), flat(W4[:, c]), flat(W5[:, c]),
                        1.0, OP.mult, OP.add)
                pop_pending()

            hf = hseq[1 + L]
            nc.sync.dma_start(oul[t], hf[:, :, :, 1])
            return hf

        # prologue: block 0's x DMA + pre-GEMM
        xb = emit_xblk_dma(0)
        cls, xpre_next = pre_gemm_closures(xb, 0)
        for cl in cls:
            cl()

        hcur, hother = HTa, HTb
        for blk in range(nblk):
            # this block's xpre tile was produced by the closures emitted
            # during the previous block (or the prologue)
            xpre_cur = xpre_next
            if blk + 1 < nblk:
                xb = emit_xblk_dma(blk + 1)
                cls, xpre_next = pre_gemm_closures(xb, blk + 1)
                pending.extend(cls)
            for j in range(unroll):
                emit_step(blk * unroll + j, xpre_cur, j, hcur, hother)
                hcur, hother = hother, hcur
            pop_pending(len(pending) if blk + 1 >= nblk else 0)

    nc.compile()
    return nc


def _install_ntff_hook_shim():
    """The agent image lacks ``antenv.axon_hooks``; recreate it and register
    trn_boot's ctypes NTFF hook so trace=True works."""
    import sys
    import types
    try:
        import antenv.axon_hooks  # noqa: F401
        return True
    except ImportError:
        pass
    try:
        import antenv
        from trn_agent_boot.trn_boot import _ntff_profile_via_ctypes
        mod = types.ModuleType('antenv.axon_hooks')
        mod._hook = _ntff_profile_via_ctypes('/opt/axon/libaxon_pjrt.so')
        mod.get_axon_ntff_profile_hook = lambda: mod._hook
        mod.set_axon_ntff_profile_hook = lambda h: setattr(mod, '_hook', h)
        sys.modules['antenv.axon_hooks'] = mod
        antenv.axon_hooks = mod
        return True
    except Exception as e:  # degrade to no-trace
        print(f'ntff hook shim failed: {e}')
        return False


def kernel(x, lengths, Wr, Wz, Wl, Wt, Cx, Ch, Tr, Tz, Tn):
    global LAST_EXEC_NS
    x = np.asarray(x)
    lengths = np.asarray(lengths)

    wp = _pack_weights(Wr, Wz, Wl, Wt, Cx, Ch, Tr, Tz, Tn)
    nc = _build_nc(T, UNROLL)

    in_maps = []
    for k in range(NCORE):
        xs = x[:, k * BS:(k + 1) * BS, :]
        in_maps.append({'xt': _pack_x_shard(xs), 'wp': wp})

    trace = bool(int(os.environ.get('RNN_KERNEL_TRACE', '0')))
    if trace:
        trace = _install_ntff_hook_shim()
    res = run_bass_kernel_spmd(nc, in_maps, core_ids=list(range(NCORE)),
                               trace=trace)
    LAST_EXEC_NS = res.exec_time_ns

    out = np.empty((T, B, H), np.float32)
    for k in range(NCORE):
        o = np.asarray(res.results[k]['out'], np.float32)  # [T,128,KC,BS]
        out[:, k * BS:(k + 1) * BS, :] = o.transpose(0, 3, 2, 1).reshape(T, BS, H)
    mask = np.arange(T)[:, None] < lengths[None, :]
    out *= mask[:, :, None].astype(np.float32)
    return out


# revision 9
# speedup vs baseline: 3.9430x; 3.9430x over previous
"""Trainium2 Bass kernel for nn_DeepTransitionRNN_31928786878509.

kernel(**inputs) -> np.ndarray, matching reference.reference semantics:
a deep-transition GRU over T=512 steps, B=128 (packed-sequence masking),
D=H=256, L=4 transition layers.

Strategy: data-parallel over batch (16 rows/core on 8 cores). Each core runs
the full 512-step recurrence with the hidden state resident in SBUF in
transposed layout h^T [feat partitions x batch], weights stationary as fp16
[128,128] chunks, PSUM fp32 accumulation, sigmoid/tanh on the scalar engine.

v3 changes vs the 5.7ms baseline (which was latency-bound on the per-sub-
layer ACT/DVE tail chains, with the PE idle ~55% of each step):
  - x-projections (Wr_x/Wz_x/Wl_x/Cx/Wt) are hoisted out of the recurrence
    into per-8-step-block pre-GEMMs at moving N=128 (vs N=16 in-loop),
    interleaved into the recurrence's PE-idle gaps; their r/z/l results are
    injected into PSUM via one identity matmul per step, and Cx/Wt results
    are consumed straight from SBUF by the tail ops.
  - per-layer matmul order rr -> zz -> Tn and ACT emission order
    sigma(rr), tanh, sigma(zz) so both sigmoids hide under PE/DVE work.
  - the final blend scan is split per feature chunk so the next sub-layer's
    kc=0 matmuls overlap the second half-scan.
  - the output transpose (2 PE pairs) + DVE copy per step are replaced by a
    direct strided DMA of the hidden tile; the de-transpose happens on host.
Packed-sequence masking (out=0 for t >= lengths[b]) is applied on the host.
"""

import os
import numpy as np
from contextlib import ExitStack

import concourse.bass as bass
import concourse.bacc as bacc
import concourse.mybir as mybir
import concourse.tile as tile
from concourse.bass_utils import run_bass_kernel_spmd

f16 = mybir.dt.float16
f32 = mybir.dt.float32
AF = mybir.ActivationFunctionType
OP = mybir.AluOpType

T, B, D, H, L = 512, 128, 256, 256, 4
NCORE = 8
BS = B // NCORE
KC_D = D // 128
KC_H = H // 128
MC = H // 128
NCH = 3 * 4 * MC + 3 * KC_H * MC + 3 * L * KC_H * MC  # 84

UNROLL = 8

LAST_EXEC_NS = None  # set by kernel() when tracing is enabled


def _pack_weights(Wr, Wz, Wl, Wt, Cx, Ch, Tr, Tz, Tn):
    chunks = []

    def add(M):
        for kc in range(M.shape[0] // 128):
            for mc in range(MC):
                chunks.append(M[kc * 128:(kc + 1) * 128, mc * 128:(mc + 1) * 128])

    add(Wr); add(Wz); add(Wl); add(Cx); add(Wt); add(Ch)
    for i in range(L):
        add(Tr[i]); add(Tz[i]); add(Tn[i])
    arr = np.stack([np.asarray(c, dtype=np.float32) for c in chunks])
    arr = arr.transpose(1, 0, 2).astype(np.float16)
    ident = np.eye(128, dtype=np.float16)[:, None, :]
    arr = np.ascontiguousarray(np.concatenate([arr, ident], axis=1))
    return arr


def _pack_x_shard(x_shard):
    # [T, BS, D] -> [nblk, 128, KC_D, UNROLL*BS] fp16: per-block matmul rhs
    # with partition = feature-in-chunk and moving cols = (t, b)
    Tn_ = x_shard.shape[0]
    nblk = Tn_ // UNROLL
    y = np.asarray(x_shard, dtype=np.float16).reshape(
        nblk, UNROLL, BS, KC_D, 128)
    y = y.transpose(0, 4, 3, 1, 2)  # [nblk, 128, KC_D, UNROLL, BS]
    return np.ascontiguousarray(y.reshape(nblk, 128, KC_D, UNROLL * BS))


def _build_nc(Tsteps, unroll):
    assert Tsteps % unroll == 0
    nblk = Tsteps // unroll
    nc = bacc.Bacc(None, target_bir_lowering=False, debug=False)
    xin = nc.dram_tensor('xt', [Tsteps // unroll, 128, KC_D, unroll * BS], f16,
                         kind='ExternalInput')
    win = nc.dram_tensor('wp', [128, NCH + 1, 128], f16, kind='ExternalInput')
    oul = nc.dram_tensor('out', [Tsteps // unroll, 128, KC_H * unroll * BS], f16,
                         kind='ExternalOutput')

    idx = {}
    pos = 0

    def reg(name, kt):
        nonlocal pos
        idx[name] = [[pos + kc * MC + mc for mc in range(MC)] for kc in range(kt)]
        pos += kt * MC

    reg('Wr', 4); reg('Wz', 4); reg('Wl', 4)
    reg('Cx', 2); reg('Wt', 2); reg('Ch', 2)
    for i in range(L):
        reg(f'Tr{i}', 2); reg(f'Tz{i}', 2); reg(f'Tn{i}', 2)
    assert pos == NCH
    ID_CHUNK = NCH

    # pre-GEMM slot order in ps_pre / xpre tiles: (r,l,z) x mc then (cx,wt) x mc
    PRE = ([('Wr', mc) for mc in range(MC)] + [('Wl', mc) for mc in range(MC)]
           + [('Wz', mc) for mc in range(MC)] + [('Cx', mc) for mc in range(MC)]
           + [('Wt', mc) for mc in range(MC)])
    NPRE = len(PRE)  # 10

    with ExitStack() as ctx:
        tc = ctx.enter_context(tile.TileContext(nc))
        wpool = ctx.enter_context(tc.tile_pool(name='w', bufs=1))
        hpool = ctx.enter_context(tc.tile_pool(name='h', bufs=1))
        spool = ctx.enter_context(tc.tile_pool(name='s', bufs=2))
        xpool = ctx.enter_context(tc.tile_pool(name='x', bufs=2))
        xppool = ctx.enter_context(tc.tile_pool(name='xp', bufs=2))
        opool = ctx.enter_context(tc.tile_pool(name='o', bufs=2))
        ps_rzl_pool = ctx.enter_context(tc.tile_pool(name='ps_rzl', bufs=1, space='PSUM'))
        ps_ch_pool = ctx.enter_context(tc.tile_pool(name='ps_ch', bufs=1, space='PSUM'))
        ps_rz_pool = ctx.enter_context(tc.tile_pool(name='ps_rz', bufs=1, space='PSUM'))
        ps_n_pool = ctx.enter_context(tc.tile_pool(name='ps_n', bufs=1, space='PSUM'))
        ps_pre_pool = ctx.enter_context(tc.tile_pool(name='ps_pre', bufs=1, space='PSUM'))

        W = wpool.tile([128, NCH + 1, 128], f16)
        nc.gpsimd.dma_start(W[:], win[:])

        HTa = hpool.tile([128, KC_H, BS, 3], f16, tag='hta')
        HTb = hpool.tile([128, KC_H, BS, 3], f16, tag='htb')
        nc.gpsimd.memset(HTa[:], 0.0)
        nc.gpsimd.memset(HTb[:], 0.0)
        W4 = hpool.tile([128, KC_H, BS, 3], f32, tag='w4')
        W5 = hpool.tile([128, KC_H, BS, 3], f32, tag='w5')
        nc.gpsimd.memset(W4[:], 0.0)
        nc.gpsimd.memset(W5[:], 0.0)
        nc.gpsimd.memset(W5[:, :, :, 2], 1.0)

        def flat(ap):
            return ap.rearrange('p b j -> p (b j)')

        def mm(out_ap, name, kc, mc, rhs, start, stop):
            nc.tensor.matmul(out_ap, W[:, idx[name][kc][mc], :], rhs,
                             start=start, stop=stop)

        # ---- x pre-GEMM machinery -------------------------------------
        def emit_xblk_dma(blk):
            xb = xpool.tile([128, KC_D, unroll * BS], f16, tag='xb')
            nc.sync.dma_start(xb[:], xin[blk])
            return xb

        def pre_gemm_closures(xb, blk):
            """Closures emitting the per-block x pre-GEMM: 20 matmul pairs
            (N=128) + a PSUM->SBUF DMA. Returns (closures, xpre_tile)."""
            ps_pre = ps_pre_pool.tile([128, NPRE, unroll * BS], f32, tag='pre')
            xpre = xppool.tile([128, NPRE, unroll, BS], f16, tag='xpre',
                               name=f'xpre_{blk}')
            cls = []
            for si, (g, mc) in enumerate(PRE):
                def do(si=si, g=g, mc=mc):
                    for kc in range(KC_D):
                        mm(ps_pre[:, si], g, kc, mc, xb[:, kc],
                           kc == 0, kc == KC_D - 1)
                    nc.vector.tensor_copy(
                        xpre[:, si].rearrange('p t b -> p (t b)'),
                        ps_pre[:, si])
                cls.append(do)
            return cls, xpre

        pending = []

        def pop_pending(k=1):
            for _ in range(k):
                if pending:
                    pending.pop(0)()

        def emit_step(ob, xpre, j, hcur, hother):
            # hseq[s] = tile holding h BEFORE sub-layer s (s=0 is the cell).
            hseq = [hcur if s % 2 == 0 else hother for s in range(2 + L)]

            def h_ap(tile_, kc):
                return tile_[:, kc, :, 1]

            h0 = hseq[0]
            ps_rzl = ps_rzl_pool.tile([128, 3, MC, BS], f32, tag='ps_rzl')
            ps_ch = ps_ch_pool.tile([128, MC, BS], f32, tag='ps_ch')

            # inject x-part of r/l/z preacts (one identity matmul), then
            # accumulate h-parts. r first so sigma(r) is ready early.
            nc.tensor.matmul(ps_rzl[:].rearrange('p g m b -> p (g m b)'),
                             W[:, ID_CHUNK, :], xpre[:, 0:6, j],
                             start=True, stop=False)
            for mc in range(MC):
                for kc in range(KC_H):
                    mm(ps_rzl[:, 0, mc], 'Wr', KC_D + kc, mc, h_ap(h0, kc), False,
                       mc == MC - 1 and kc == KC_H - 1)
            for mc in range(MC):
                for kc in range(KC_H):
                    mm(ps_ch[:, mc], 'Ch', kc, mc, h_ap(h0, kc),
                       mc == 0 and kc == 0, mc == MC - 1 and kc == KC_H - 1)
            for gi, g in ((1, 'Wl'), (2, 'Wz')):
                for mc in range(MC):
                    for kc in range(KC_H):
                        mm(ps_rzl[:, gi, mc], g, KC_D + kc, mc, h_ap(h0, kc), False,
                           gi == 2 and mc == MC - 1 and kc == KC_H - 1)
            pop_pending()

            s_r = spool.tile([128, MC, BS], f32, tag='s_r')
            nc.scalar.activation(s_r[:], ps_rzl[:, 0], AF.Sigmoid)
            s_l = spool.tile([128, MC, BS], f32, tag='s_l')
            nc.scalar.activation(s_l[:], ps_rzl[:, 1], AF.Sigmoid)
            u = spool.tile([128, MC, BS], f32, tag='u')
            nc.vector.tensor_tensor(u[:], s_r[:], ps_ch[:], OP.mult)
            # v = u + xCx lands in ps_ch (consumed) for the faster ACT PSUM read
            nc.vector.tensor_tensor(ps_ch[:], u[:], xpre[:, 6:8, j], OP.add)
            nnc = spool.tile([128, MC, BS], f32, tag='nnc')
            nc.scalar.activation(nnc[:], ps_ch[:], AF.Tanh)
            # 1 - sig(zpre) = sig(-zpre): cell blend h' = sig(-z)*(h - n) + n
            nc.scalar.activation(W4[:, :, :, 1], ps_rzl[:, 2], AF.Sigmoid,
                                 scale=-1.0)
            w_ = spool.tile([128, MC, BS], f32, tag='w_')
            nc.vector.tensor_tensor(w_[:], s_l[:], xpre[:, 8:10, j], OP.mult)
            nc.vector.tensor_tensor(W5[:, :, :, 1], nnc[:], w_[:], OP.add)
            nc.vector.tensor_tensor(W4[:, :, :, 0], h0[:, :, :, 1],
                                    W5[:, :, :, 1], OP.subtract)
            for c in range(KC_H):
                nc.vector.tensor_tensor_scan(
                    flat(hseq[1][:, c]), flat(W4[:, c]), flat(W5[:, c]), 1.0,
                    OP.mult, OP.add)
            pop_pending()

            for li in range(L):
                hp = hseq[1 + li]
                ps_rz = ps_rz_pool.tile([128, 2, MC, BS], f32, tag='ps_rz')
                ps_n = ps_n_pool.tile([128, MC, BS], f32, tag='ps_n')
                # rr first (its sigmoid gates the critical path), then zz,
                # then Tn; sigma(rr) hides under the zz/Tn matmuls.
                for mc in range(MC):
                    for kc in range(KC_H):
                        mm(ps_rz[:, 0, mc], f'Tr{li}', kc, mc, h_ap(hp, kc),
                           mc == 0 and kc == 0, False)
                for mc in range(MC):
                    for kc in range(KC_H):
                        mm(ps_rz[:, 1, mc], f'Tz{li}', kc, mc, h_ap(hp, kc), False,
                           mc == MC - 1 and kc == KC_H - 1)
                for mc in range(MC):
                    for kc in range(KC_H):
                        mm(ps_n[:, mc], f'Tn{li}', kc, mc, h_ap(hp, kc),
                           mc == 0 and kc == 0, mc == MC - 1 and kc == KC_H - 1)
                s_rr = spool.tile([128, MC, BS], f32, tag='s_rr')
                nc.scalar.activation(s_rr[:], ps_rz[:, 0], AF.Sigmoid)
                # m = sig(rr) * (h@Tn) lands back in ps_n for ACT PSUM read
                nc.vector.tensor_tensor(ps_n[:], s_rr[:], ps_n[:], OP.mult)
                nc.scalar.activation(W5[:, :, :, 1], ps_n[:], AF.Tanh)
                nc.scalar.activation(W4[:, :, :, 1], ps_rz[:, 1], AF.Sigmoid)
                nc.vector.tensor_tensor(W4[:, :, :, 0], hp[:, :, :, 1],
                                        W5[:, :, :, 1], OP.subtract)
                # h' = sig(zz)*(h - nn) + nn, split per chunk so the next
                # sub-layer's kc=0 matmuls overlap the second half-scan
                for c in range(KC_H):
                    nc.vector.tensor_tensor_scan(
                        flat(hseq[2 + li][:, c]), flat(W4[:, c]), flat(W5[:, c]),
                        1.0, OP.mult, OP.add)
                pop_pending()

            hf = hseq[1 + L]
            nc.gpsimd.tensor_copy(ob[:, :, j, :], hf[:, :, :, 1])
            return hf

        # prologue: block 0's x DMA + pre-GEMM
        xb = emit_xblk_dma(0)
        cls, xpre_next = pre_gemm_closures(xb, 0)
        for cl in cls:
            cl()

        hcur, hother = HTa, HTb
        for blk in range(nblk):
            # this block's xpre tile was produced by the closures emitted
            # during the previous block (or the prologue)
            xpre_cur = xpre_next
            if blk + 1 < nblk:
                xb = emit_xblk_dma(blk + 1)
                cls, xpre_next = pre_gemm_closures(xb, blk + 1)
                pending.extend(cls)
            ob = opool.tile([128, KC_H, unroll, BS], f16, tag='ob')
            for j in range(unroll):
                emit_step(ob, xpre_cur, j, hcur, hother)
                hcur, hother = hother, hcur
            nc.sync.dma_start(
                oul[blk], ob[:].rearrange('p c t b -> p (c t b)'))
            pop_pending(len(pending) if blk + 1 >= nblk else 0)

    nc.compile()
    return nc


def _install_ntff_hook_shim():
    """The agent image lacks ``antenv.axon_hooks``; recreate it and register
    trn_boot's ctypes NTFF hook so trace=True works."""
    import sys
    import types
    try:
        import antenv.axon_hooks  # noqa: F401
        return True
    except ImportError:
        pass
    try:
        import antenv
        from trn_agent_boot.trn_boot import _ntff_profile_via_ctypes
        mod = types.ModuleType('antenv.axon_hooks')
        mod._hook = _ntff_profile_via_ctypes('/opt/axon/libaxon_pjrt.so')
        mod.get_axon_ntff_profile_hook = lambda: mod._hook
        mod.set_axon_ntff_profile_hook = lambda h: setattr(mod, '_hook', h)
        sys.modules['antenv.axon_hooks'] = mod
        antenv.axon_hooks = mod
        return True
    except Exception as e:  # degrade to no-trace
        print(f'ntff hook shim failed: {e}')
        return False


def kernel(x, lengths, Wr, Wz, Wl, Wt, Cx, Ch, Tr, Tz, Tn):
    global LAST_EXEC_NS
    x = np.asarray(x)
    lengths = np.asarray(lengths)

    wp = _pack_weights(Wr, Wz, Wl, Wt, Cx, Ch, Tr, Tz, Tn)
    nc = _build_nc(T, UNROLL)

    in_maps = []
    for k in range(NCORE):
        xs = x[:, k * BS:(k + 1) * BS, :]
        in_maps.append({'xt': _pack_x_shard(xs), 'wp': wp})

    trace = bool(int(os.environ.get('RNN_KERNEL_TRACE', '0')))
    if trace:
        trace = _install_ntff_hook_shim()
    res = run_bass_kernel_spmd(nc, in_maps, core_ids=list(range(NCORE)),
                               trace=trace)
    LAST_EXEC_NS = res.exec_time_ns

    out = np.empty((T, B, H), np.float32)
    for k in range(NCORE):
        o = np.asarray(res.results[k]['out'], np.float32).reshape(
            T // UNROLL, 128, KC_H, UNROLL, BS)
        # [blk, p, c, t, b] -> [blk, t, b, c, p] -> [T, BS, H]
        out[:, k * BS:(k + 1) * BS, :] = o.transpose(0, 3, 4, 2, 1).reshape(
            T, BS, H)
    mask = np.arange(T)[:, None] < lengths[None, :]
    out *= mask[:, :, None].astype(np.float32)
    return out


# revision 10
# speedup vs baseline: 4.1732x; 1.0584x over previous
"""Trainium2 Bass kernel for nn_DeepTransitionRNN_31928786878509.

kernel(**inputs) -> np.ndarray, matching reference.reference semantics:
a deep-transition GRU over T=512 steps, B=128 (packed-sequence masking),
D=H=256, L=4 transition layers.

Strategy: data-parallel over batch (16 rows/core on 8 cores). Each core runs
the full 512-step recurrence with the hidden state resident in SBUF in
transposed layout h^T [feat partitions x batch], weights stationary as fp16
[128,128] chunks, PSUM fp32 accumulation, sigmoid/tanh on the scalar engine.

v3 changes vs the 5.7ms baseline (which was latency-bound on the per-sub-
layer ACT/DVE tail chains, with the PE idle ~55% of each step):
  - x-projections (Wr_x/Wz_x/Wl_x/Cx/Wt) are hoisted out of the recurrence
    into per-8-step-block pre-GEMMs at moving N=128 (vs N=16 in-loop),
    interleaved into the recurrence's PE-idle gaps; their r/z/l results are
    injected into PSUM via one identity matmul per step, and Cx/Wt results
    are consumed straight from SBUF by the tail ops.
  - per-layer matmul order rr -> zz -> Tn and ACT emission order
    sigma(rr), tanh, sigma(zz) so both sigmoids hide under PE/DVE work.
  - the final blend scan is split per feature chunk so the next sub-layer's
    kc=0 matmuls overlap the second half-scan.
  - the output transpose (2 PE pairs) + DVE copy per step are replaced by a
    direct strided DMA of the hidden tile; the de-transpose happens on host.
Packed-sequence masking (out=0 for t >= lengths[b]) is applied on the host.
"""

import os
import numpy as np
from contextlib import ExitStack

import concourse.bass as bass
import concourse.bacc as bacc
import concourse.mybir as mybir
import concourse.tile as tile
from concourse.bass_utils import run_bass_kernel_spmd

f16 = mybir.dt.float16
f32 = mybir.dt.float32
AF = mybir.ActivationFunctionType
OP = mybir.AluOpType

T, B, D, H, L = 512, 128, 256, 256, 4
NCORE = 8
BS = B // NCORE
KC_D = D // 128
KC_H = H // 128
MC = H // 128
NCH = 3 * 4 * MC + 3 * KC_H * MC + 3 * L * KC_H * MC  # 84

UNROLL = 8

LAST_EXEC_NS = None  # set by kernel() when tracing is enabled


def _pack_weights(Wr, Wz, Wl, Wt, Cx, Ch, Tr, Tz, Tn):
    chunks = []

    def add(M):
        for kc in range(M.shape[0] // 128):
            for mc in range(MC):
                chunks.append(M[kc * 128:(kc + 1) * 128, mc * 128:(mc + 1) * 128])

    add(Wr); add(Wz); add(Wl); add(Cx); add(Wt); add(Ch)
    for i in range(L):
        add(Tr[i]); add(Tz[i]); add(Tn[i])
    arr = np.stack([np.asarray(c, dtype=np.float32) for c in chunks])
    arr = arr.transpose(1, 0, 2).astype(np.float16)
    ident = np.eye(128, dtype=np.float16)[:, None, :]
    arr = np.ascontiguousarray(np.concatenate([arr, ident], axis=1))
    return arr


def _pack_x_shard(x_shard):
    # [T, BS, D] -> [nblk, 128, KC_D, UNROLL*BS] fp16: per-block matmul rhs
    # with partition = feature-in-chunk and moving cols = (t, b)
    Tn_ = x_shard.shape[0]
    nblk = Tn_ // UNROLL
    y = np.asarray(x_shard, dtype=np.float16).reshape(
        nblk, UNROLL, BS, KC_D, 128)
    y = y.transpose(0, 4, 3, 1, 2)  # [nblk, 128, KC_D, UNROLL, BS]
    return np.ascontiguousarray(y.reshape(nblk, 128, KC_D, UNROLL * BS))


def _build_nc(Tsteps, unroll):
    assert Tsteps % unroll == 0
    nblk = Tsteps // unroll
    nc = bacc.Bacc(None, target_bir_lowering=False, debug=False)
    xin = nc.dram_tensor('xt', [Tsteps // unroll, 128, KC_D, unroll * BS], f16,
                         kind='ExternalInput')
    win = nc.dram_tensor('wp', [128, NCH + 1, 128], f16, kind='ExternalInput')
    oul = nc.dram_tensor('out', [Tsteps // unroll, 128, KC_H * unroll * BS], f16,
                         kind='ExternalOutput')

    idx = {}
    pos = 0

    def reg(name, kt):
        nonlocal pos
        idx[name] = [[pos + kc * MC + mc for mc in range(MC)] for kc in range(kt)]
        pos += kt * MC

    reg('Wr', 4); reg('Wz', 4); reg('Wl', 4)
    reg('Cx', 2); reg('Wt', 2); reg('Ch', 2)
    for i in range(L):
        reg(f'Tr{i}', 2); reg(f'Tz{i}', 2); reg(f'Tn{i}', 2)
    assert pos == NCH
    ID_CHUNK = NCH

    # pre-GEMM slot order in ps_pre / xpre tiles: (r,l,z) x mc then (cx,wt) x mc
    PRE = ([('Wr', mc) for mc in range(MC)] + [('Wl', mc) for mc in range(MC)]
           + [('Wz', mc) for mc in range(MC)] + [('Cx', mc) for mc in range(MC)]
           + [('Wt', mc) for mc in range(MC)])
    NPRE = len(PRE)  # 10

    with ExitStack() as ctx:
        tc = ctx.enter_context(tile.TileContext(nc))
        wpool = ctx.enter_context(tc.tile_pool(name='w', bufs=1))
        hpool = ctx.enter_context(tc.tile_pool(name='h', bufs=1))
        spool = ctx.enter_context(tc.tile_pool(name='s', bufs=2))
        xpool = ctx.enter_context(tc.tile_pool(name='x', bufs=2))
        xppool = ctx.enter_context(tc.tile_pool(name='xp', bufs=2))
        opool = ctx.enter_context(tc.tile_pool(name='o', bufs=2))
        ps_r_pool = ctx.enter_context(tc.tile_pool(name='ps_r', bufs=1, space='PSUM'))
        ps_lz_pool = ctx.enter_context(tc.tile_pool(name='ps_lz', bufs=1, space='PSUM'))
        ps_ch_pool = ctx.enter_context(tc.tile_pool(name='ps_ch', bufs=1, space='PSUM'))
        ps_rr_pool = ctx.enter_context(tc.tile_pool(name='ps_rr', bufs=1, space='PSUM'))
        ps_z_pool = ctx.enter_context(tc.tile_pool(name='ps_z', bufs=1, space='PSUM'))
        ps_n_pool = ctx.enter_context(tc.tile_pool(name='ps_n', bufs=1, space='PSUM'))
        ps_pre_pool = ctx.enter_context(tc.tile_pool(name='ps_pre', bufs=1, space='PSUM'))

        W = wpool.tile([128, NCH + 1, 128], f16)
        nc.gpsimd.dma_start(W[:], win[:])

        HTa = hpool.tile([128, KC_H, BS, 3], f16, tag='hta')
        HTb = hpool.tile([128, KC_H, BS, 3], f16, tag='htb')
        nc.gpsimd.memset(HTa[:], 0.0)
        nc.gpsimd.memset(HTb[:], 0.0)
        W4 = hpool.tile([128, KC_H, BS, 3], f32, tag='w4')
        W5 = hpool.tile([128, KC_H, BS, 3], f32, tag='w5')
        nc.gpsimd.memset(W4[:], 0.0)
        nc.gpsimd.memset(W5[:], 0.0)
        nc.gpsimd.memset(W5[:, :, :, 2], 1.0)

        def flat(ap):
            return ap.rearrange('p b j -> p (b j)')

        def mm(out_ap, name, kc, mc, rhs, start, stop):
            nc.tensor.matmul(out_ap, W[:, idx[name][kc][mc], :], rhs,
                             start=start, stop=stop)

        # ---- x pre-GEMM machinery -------------------------------------
        def emit_xblk_dma(blk):
            xb = xpool.tile([128, KC_D, unroll * BS], f16, tag='xb')
            nc.sync.dma_start(xb[:], xin[blk])
            return xb

        def pre_gemm_closures(xb, blk):
            """Closures emitting the per-block x pre-GEMM: 20 matmul pairs
            (N=128) + a PSUM->SBUF DMA. Returns (closures, xpre_tile)."""
            ps_pre = ps_pre_pool.tile([128, 5, unroll * BS], f32, tag='pre')
            xpre = xppool.tile([128, NPRE, unroll, BS], f16, tag='xpre',
                               name=f'xpre_{blk}')
            cls = []
            for si, (g, mc) in enumerate(PRE):
                def do(si=si, g=g, mc=mc):
                    for kc in range(KC_D):
                        mm(ps_pre[:, si % 5], g, kc, mc, xb[:, kc],
                           kc == 0, kc == KC_D - 1)
                    nc.vector.tensor_copy(
                        xpre[:, si].rearrange('p t b -> p (t b)'),
                        ps_pre[:, si % 5])
                cls.append(do)
            return cls, xpre

        pending = []

        def pop_pending(k=1):
            for _ in range(k):
                if pending:
                    pending.pop(0)()

        def emit_step(ob, xpre, j, hcur, hother):
            # hseq[s] = tile holding h BEFORE sub-layer s (s=0 is the cell).
            hseq = [hcur if s % 2 == 0 else hother for s in range(2 + L)]

            def h_ap(tile_, kc):
                return tile_[:, kc, :, 1]

            h0 = hseq[0]
            ps_r = ps_r_pool.tile([128, MC, BS], f32, tag='ps_r')
            ps_lz = ps_lz_pool.tile([128, 2, MC, BS], f32, tag='ps_lz')
            ps_ch = ps_ch_pool.tile([128, MC, BS], f32, tag='ps_ch')

            # inject x-part of r preacts (identity matmul), then accumulate
            # h-parts; r has its own psum tile so its group closes (and
            # sigma(r) starts) before the l/z/Ch matmuls finish.
            nc.tensor.matmul(ps_r[:].rearrange('p m b -> p (m b)'),
                             W[:, ID_CHUNK, :], xpre[:, 0:2, j],
                             start=True, stop=False)
            for mc in range(MC):
                for kc in range(KC_H):
                    mm(ps_r[:, mc], 'Wr', KC_D + kc, mc, h_ap(h0, kc), False,
                       mc == MC - 1 and kc == KC_H - 1)
            for mc in range(MC):
                for kc in range(KC_H):
                    mm(ps_ch[:, mc], 'Ch', kc, mc, h_ap(h0, kc),
                       mc == 0 and kc == 0, mc == MC - 1 and kc == KC_H - 1)
            nc.tensor.matmul(ps_lz[:].rearrange('p g m b -> p (g m b)'),
                             W[:, ID_CHUNK, :], xpre[:, 2:6, j],
                             start=True, stop=False)
            for gi, g in ((0, 'Wl'), (1, 'Wz')):
                for mc in range(MC):
                    for kc in range(KC_H):
                        mm(ps_lz[:, gi, mc], g, KC_D + kc, mc, h_ap(h0, kc), False,
                           gi == 1 and mc == MC - 1 and kc == KC_H - 1)
            pop_pending()

            s_r = spool.tile([128, MC, BS], f32, tag='s_r')
            nc.scalar.activation(s_r[:], ps_r[:], AF.Sigmoid)
            s_l = spool.tile([128, MC, BS], f32, tag='s_l')
            nc.scalar.activation(s_l[:], ps_lz[:, 0], AF.Sigmoid)
            u = spool.tile([128, MC, BS], f32, tag='u')
            nc.vector.tensor_tensor(u[:], s_r[:], ps_ch[:], OP.mult)
            # v = u + xCx lands in ps_ch (consumed) for the faster ACT PSUM read
            nc.vector.tensor_tensor(ps_ch[:], u[:], xpre[:, 6:8, j], OP.add)
            nnc = spool.tile([128, MC, BS], f32, tag='nnc')
            nc.scalar.activation(nnc[:], ps_ch[:], AF.Tanh)
            # 1 - sig(zpre) = sig(-zpre): cell blend h' = sig(-z)*(h - n) + n
            nc.scalar.activation(W4[:, :, :, 1], ps_lz[:, 1], AF.Sigmoid,
                                 scale=-1.0)
            w_ = spool.tile([128, MC, BS], f32, tag='w_')
            nc.vector.tensor_tensor(w_[:], s_l[:], xpre[:, 8:10, j], OP.mult)
            nc.vector.tensor_tensor(W5[:, :, :, 1], nnc[:], w_[:], OP.add)
            nc.vector.tensor_tensor(W4[:, :, :, 0], h0[:, :, :, 1],
                                    W5[:, :, :, 1], OP.subtract)
            for c in range(KC_H):
                nc.vector.tensor_tensor_scan(
                    flat(hseq[1][:, c]), flat(W4[:, c]), flat(W5[:, c]), 1.0,
                    OP.mult, OP.add)
            pop_pending()

            for li in range(L):
                hp = hseq[1 + li]
                ps_rr = ps_rr_pool.tile([128, MC, BS], f32, tag='ps_rr')
                ps_z = ps_z_pool.tile([128, MC, BS], f32, tag='ps_z')
                ps_n = ps_n_pool.tile([128, MC, BS], f32, tag='ps_n')
                # rr first in its own tile: its group closes after 4 matmuls
                # so sigma(rr) runs under the zz/Tn matmuls.
                for mc in range(MC):
                    for kc in range(KC_H):
                        mm(ps_rr[:, mc], f'Tr{li}', kc, mc, h_ap(hp, kc),
                           mc == 0 and kc == 0, mc == MC - 1 and kc == KC_H - 1)
                for mc in range(MC):
                    for kc in range(KC_H):
                        mm(ps_z[:, mc], f'Tz{li}', kc, mc, h_ap(hp, kc),
                           mc == 0 and kc == 0, mc == MC - 1 and kc == KC_H - 1)
                for mc in range(MC):
                    for kc in range(KC_H):
                        mm(ps_n[:, mc], f'Tn{li}', kc, mc, h_ap(hp, kc),
                           mc == 0 and kc == 0, mc == MC - 1 and kc == KC_H - 1)
                s_rr = spool.tile([128, MC, BS], f32, tag='s_rr')
                nc.scalar.activation(s_rr[:], ps_rr[:], AF.Sigmoid)
                # m = sig(rr) * (h@Tn) lands back in ps_n for ACT PSUM read
                nc.vector.tensor_tensor(ps_n[:], s_rr[:], ps_n[:], OP.mult)
                nc.scalar.activation(W5[:, :, :, 1], ps_n[:], AF.Tanh)
                nc.scalar.activation(W4[:, :, :, 1], ps_z[:], AF.Sigmoid)
                nc.vector.tensor_tensor(W4[:, :, :, 0], hp[:, :, :, 1],
                                        W5[:, :, :, 1], OP.subtract)
                # h' = sig(zz)*(h - nn) + nn, split per chunk so the next
                # sub-layer's kc=0 matmuls overlap the second half-scan
                for c in range(KC_H):
                    nc.vector.tensor_tensor_scan(
                        flat(hseq[2 + li][:, c]), flat(W4[:, c]), flat(W5[:, c]),
                        1.0, OP.mult, OP.add)
                pop_pending()

            hf = hseq[1 + L]
            nc.gpsimd.tensor_copy(ob[:, :, j, :], hf[:, :, :, 1])
            return hf

        # prologue: block 0's x DMA + pre-GEMM
        xb = emit_xblk_dma(0)
        cls, xpre_next = pre_gemm_closures(xb, 0)
        for cl in cls:
            cl()

        hcur, hother = HTa, HTb
        for blk in range(nblk):
            # this block's xpre tile was produced by the closures emitted
            # during the previous block (or the prologue)
            xpre_cur = xpre_next
            if blk + 1 < nblk:
                xb = emit_xblk_dma(blk + 1)
                cls, xpre_next = pre_gemm_closures(xb, blk + 1)
                pending.extend(cls)
            ob = opool.tile([128, KC_H, unroll, BS], f16, tag='ob')
            for j in range(unroll):
                emit_step(ob, xpre_cur, j, hcur, hother)
                hcur, hother = hother, hcur
            nc.sync.dma_start(
                oul[blk], ob[:].rearrange('p c t b -> p (c t b)'))
            pop_pending(len(pending) if blk + 1 >= nblk else 0)

    nc.compile()
    return nc


def _install_ntff_hook_shim():
    """The agent image lacks ``antenv.axon_hooks``; recreate it and register
    trn_boot's ctypes NTFF hook so trace=True works."""
    import sys
    import types
    try:
        import antenv.axon_hooks  # noqa: F401
        return True
    except ImportError:
        pass
    try:
        import antenv
        from trn_agent_boot.trn_boot import _ntff_profile_via_ctypes
        mod = types.ModuleType('antenv.axon_hooks')
        mod._hook = _ntff_profile_via_ctypes('/opt/axon/libaxon_pjrt.so')
        mod.get_axon_ntff_profile_hook = lambda: mod._hook
        mod.set_axon_ntff_profile_hook = lambda h: setattr(mod, '_hook', h)
        sys.modules['antenv.axon_hooks'] = mod
        antenv.axon_hooks = mod
        return True
    except Exception as e:  # degrade to no-trace
        print(f'ntff hook shim failed: {e}')
        return False


def kernel(x, lengths, Wr, Wz, Wl, Wt, Cx, Ch, Tr, Tz, Tn):
    global LAST_EXEC_NS
    x = np.asarray(x)
    lengths = np.asarray(lengths)

    wp = _pack_weights(Wr, Wz, Wl, Wt, Cx, Ch, Tr, Tz, Tn)
    nc = _build_nc(T, UNROLL)

    in_maps = []
    for k in range(NCORE):
        xs = x[:, k * BS:(k + 1) * BS, :]
        in_maps.append({'xt': _pack_x_shard(xs), 'wp': wp})

    trace = bool(int(os.environ.get('RNN_KERNEL_TRACE', '0')))
    if trace:
        trace = _install_ntff_hook_shim()
    res = run_bass_kernel_spmd(nc, in_maps, core_ids=list(range(NCORE)),
                               trace=trace)
    LAST_EXEC_NS = res.exec_time_ns

    out = np.empty((T, B, H), np.float32)
    for k in range(NCORE):
        o = np.asarray(res.results[k]['out'], np.float32).reshape(
            T // UNROLL, 128, KC_H, UNROLL, BS)
        # [blk, p, c, t, b] -> [blk, t, b, c, p] -> [T, BS, H]
        out[:, k * BS:(k + 1) * BS, :] = o.transpose(0, 3, 4, 2, 1).reshape(
            T, BS, H)
    mask = np.arange(T)[:, None] < lengths[None, :]
    out *= mask[:, :, None].astype(np.float32)
    return out
